# revision 3
# baseline (speedup 1.0000x reference)
"""Gaussian pyramid (Build_GPyr) Trainium2 Bass kernel.

Problem: im [16, 3, 1024, 1024] f32, levels=4. Output: tuple of
(im, L1, L2, L3) with L_{k+1} = 5x5 binomial blur (replicate pad) +
stride-2 downsample of L_k, depthwise per channel.

Strategy (pure data parallel, 2 images = 6 channel planes per core):
Each level applies the separable blur as two banded-matrix products on
the TensorEngine, both contracting the partition axis:

  V pass (contract rows):  lhsT = X row-tile [128 rows, 128 cols chunk]
                           rhs  = band slice W_H [128 rows, 66 out rows]
                           psum [cols chunk, out rows] += ...  -> Y^T
  H pass (contract cols):  lhsT = Y^T col-tile [128 cols, 128 out-row chunk]
                           rhs  = band slice W_W [128 cols, 66 out cols]
                           psum [out rows, out cols] += ...    -> Z (row major)

Adjacent band slices overlap by ~2 output columns; PSUM's per-element
has_written accumulate handles the overlap inside one accumulation group
(start=True on the first matmul clears the bank).

Z stays in SBUF as the next level's input and is DMA'd out per level.
"""

import numpy as np

P = 128
BW = 66  # band slab width (128 input rows reach <= 66 output rows)
NPLANES = 6  # 2 images x 3 channels per core
H0 = 1024
NCORES = 8

_CACHE = {}


def _band_tables(np_dtype):
    """Banded blur+downsample matrices, sliced per 128-row tile.

    Returns (band [128, 14*66] array, offs: H -> (slab base index, [col off per k])).
    band slab s=(base+k) column block [:, s*66:(s+1)*66] holds
    W_full[128k:128(k+1), ck:ck+66] where W_full[r, i] = sum_j w[j] *
    [clamp(2i+j-2, 0, H-1) == r].
    """
    w = np.array([1.0, 4.0, 6.0, 4.0, 1.0], np.float64) / 16.0
    slabs = []
    offs = {}
    for H in (1024, 512, 256):
        Hh = H // 2
        K = H // P
        Wf = np.zeros((H, Hh), np.float64)
        for i in range(Hh):
            for j in range(5):
                r = min(max(2 * i + j - 2, 0), H - 1)
                Wf[r, i] += w[j]
        base = len(slabs)
        cks = []
        for k in range(K):
            ck = min(max(64 * k - 1, 0), Hh - BW)
            cks.append(ck)
            slabs.append(Wf[P * k : P * (k + 1), ck : ck + BW])
        offs[H] = (base, cks)
    band = np.stack(slabs)  # [14, 128, 66]
    band = band.transpose(1, 0, 2).reshape(P, len(slabs) * BW)
    return np.ascontiguousarray(band).astype(np_dtype), offs


def _split_multi_waits(nc):
    """This walrus build rejects >1 sem wait per instruction. Hoist extra
    waits onto preceding same-engine NoOps (sequencers are in-order)."""
    import bass_rust
    import concourse.mybir as mybir

    n = 0
    for func in nc.m.functions:
        for block in func.blocks:
            out = []
            changed = False
            for inst in block.instructions:
                si = inst.sync_info
                if si is not None and si.on_wait and len(si.on_wait) > 1:
                    waits = list(si.on_wait)
                    for wt in waits[:-1]:
                        n += 1
                        nop = mybir.InstNoOp(name=f"waitsplit_{n}", ins=[], outs=[])
                        nop.engine = inst.engine
                        nop.sync_info = bass_rust.SyncInfo(on_wait=[wt], on_update=[])
                        out.append(nop)
                    si.on_wait = [waits[-1]]
                    changed = True
                out.append(inst)
            if changed:
                block.instructions = out
    return n


def _build(use_bf16, loop_reps=1):
    import concourse.bass as bass
    import concourse.mybir as mybir
    from concourse.tile import TileContext

    if use_bf16:
        import ml_dtypes

        np_in = ml_dtypes.bfloat16
        dt_in = mybir.dt.bfloat16
    else:
        np_in = np.float32
        dt_in = mybir.dt.float32
    f32 = mybir.dt.float32

    band_np, offs = _band_tables(np_in)
    nbands = band_np.shape[1] // BW

    nc = bass.Bass(enable_partition_id=False)
    x = nc.dram_tensor("x", [NPLANES, H0, H0], dt_in, kind="ExternalInput")
    wb = nc.dram_tensor("wb", [P, nbands * BW], dt_in, kind="ExternalInput")
    louts = {
        1: nc.dram_tensor("l1", [NPLANES, 512, 512], f32, kind="ExternalOutput"),
        2: nc.dram_tensor("l2", [NPLANES, 256, 256], f32, kind="ExternalOutput"),
        3: nc.dram_tensor("l3", [NPLANES, 128, 128], f32, kind="ExternalOutput"),
    }

    copy_ctr = [0]

    with TileContext(nc) as tc:
        with (
            tc.tile_pool(name="consts", bufs=1) as cpool,
            tc.tile_pool(name="sbuf", bufs=2) as pool,
            tc.tile_pool(name="psum", bufs=3, space="PSUM") as ppool,
        ):
            wbt = cpool.tile([P, nbands * BW], dt_in, name="wbt")
            nc.sync.dma_start(out=wbt, in_=wb[:, :])

            def band_slice(H, k):
                base, _ = offs[H]
                s = base + k
                return wbt[:, s * BW : (s + 1) * BW]

            def copy(dst, src):
                # alternate PSUM->SBUF evacuation between ScalarE and VectorE
                copy_ctr[0] += 1
                if copy_ctr[0] % 2 == 0:
                    nc.scalar.copy(out=dst, in_=src)
                else:
                    nc.vector.tensor_copy(out=dst, in_=src)

            def emit_plane(p):
                xt = pool.tile([P, H0 // P, H0], dt_in, name="xt", tag="xt", bufs=2)
                nc.sync.dma_start(
                    out=xt, in_=x[p].rearrange("(k q) w -> q k w", q=P)
                )

                cur = xt  # [128, K, W] row-tile layout of current level input
                H = H0
                for lvl in (1, 2, 3):
                    Hh = H // 2
                    K = H // P  # input row tiles == input col chunks
                    Q = Hh // P  # output row chunks
                    _, cks = offs[H]

                    # V pass: Y^T[c, i] per 128-col chunk m
                    yt = pool.tile(
                        [P, K, Hh], dt_in, name=f"yt{lvl}", tag=f"yt{lvl}", bufs=2
                    )
                    for m in range(K):
                        ps = ppool.tile(
                            [P, Hh], f32, name=f"vps{lvl}_{m}", tag="vps", bufs=3
                        )
                        for k in range(K):
                            ck = cks[k]
                            nc.tensor.matmul(
                                ps[:, ck : ck + BW],
                                lhsT=cur[:, k, m * P : (m + 1) * P],
                                rhs=band_slice(H, k),
                                start=(k == 0),
                                stop=(k == K - 1),
                                skip_group_check=True,
                            )
                        copy(yt[:, m, :], ps)

                    # H pass: Z[i, j] per 128-out-row chunk q
                    zt = pool.tile(
                        [P, Q, Hh], dt_in, name=f"zt{lvl}", tag=f"zt{lvl}", bufs=2
                    )
                    if use_bf16:
                        ztf = pool.tile(
                            [P, Q, Hh], f32, name=f"ztf{lvl}", tag=f"ztf{lvl}", bufs=2
                        )
                    for q in range(Q):
                        ps = ppool.tile(
                            [P, Hh], f32, name=f"hps{lvl}_{q}", tag="hps", bufs=3
                        )
                        for m in range(K):
                            cm = cks[m]
                            nc.tensor.matmul(
                                ps[:, cm : cm + BW],
                                lhsT=yt[:, m, q * P : (q + 1) * P],
                                rhs=band_slice(H, m),
                                start=(m == 0),
                                stop=(m == K - 1),
                                skip_group_check=True,
                            )
                        if use_bf16:
                            copy(ztf[:, q, :], ps)
                            if lvl < 3:
                                copy(zt[:, q, :], ps)
                        else:
                            copy(zt[:, q, :], ps)

                    out_src = ztf if use_bf16 else zt
                    nc.sync.dma_start(
                        out=louts[lvl][p].rearrange("(q r) w -> r q w", r=P),
                        in_=out_src,
                    )
                    cur = zt
                    H = Hh

            if loop_reps > 1:
                with tc.For_i(0, loop_reps, 1):
                    for p in range(NPLANES):
                        emit_plane(p)
            else:
                for p in range(NPLANES):
                    emit_plane(p)

    _split_multi_waits(nc)
    return nc


def _get_nc(use_bf16):
    key = ("nc", use_bf16)
    if key not in _CACHE:
        _CACHE[key] = _build(use_bf16)
    return _CACHE[key]


def kernel(im, levels, use_bf16=False, _want_results_obj=False, **run_kwargs):
    import ml_dtypes

    levels = int(levels)
    im = np.asarray(im)
    assert im.shape == (16, 3, H0, H0), im.shape
    assert 1 <= levels <= 4, levels

    im_f32 = np.ascontiguousarray(im, dtype=np.float32)
    outs = [im_f32]
    if levels == 1:
        return tuple(outs)

    from concourse.bass_utils import run_bass_kernel_spmd

    nc = _get_nc(use_bf16)
    np_in = ml_dtypes.bfloat16 if use_bf16 else np.float32
    band_np, _ = _band_tables(np_in)

    in_maps = []
    for c in range(NCORES):
        xc = np.ascontiguousarray(
            im_f32[2 * c : 2 * c + 2].reshape(NPLANES, H0, H0)
        ).astype(np_in)
        in_maps.append({"x": xc, "wb": band_np})

    res = run_bass_kernel_spmd(nc, in_maps, core_ids=list(range(NCORES)), **run_kwargs)

    for lvl, size in ((1, 512), (2, 256), (3, 128)):
        if levels <= lvl:
            break
        full = np.empty((16, 3, size, size), np.float32)
        for c in range(NCORES):
            full[2 * c : 2 * c + 2] = res.results[c][f"l{lvl}"].reshape(
                2, 3, size, size
            )
        outs.append(full)

    if _want_results_obj:
        return tuple(outs[:levels]), res
    return tuple(outs[:levels])


# revision 8
# speedup vs baseline: 3.1793x; 3.1793x over previous
"""Gaussian pyramid (Build_GPyr) Trainium2 Bass kernel.

Problem: im [16, 3, 1024, 1024] f32, levels=4. Output: tuple of
(im, L1, L2, L3) with L_{k+1} = 5x5 binomial blur (replicate pad) +
stride-2 downsample of L_k, depthwise per channel.

Strategy (pure data parallel, 2 images = 6 channel planes per core):
Each level applies the separable blur as two banded-matrix products on
the TensorEngine, both contracting the partition axis:

  V pass (contract rows):  lhsT = X row-tile [128 rows, 128 cols chunk]
                           rhs  = band slice W_H [128 rows, 66 out rows]
                           psum [cols chunk, out rows] += ...  -> Y^T
  H pass (contract cols):  lhsT = Y^T col-tile [128 cols, 128 out-row chunk]
                           rhs  = band slice W_W [128 cols, 66 out cols]
                           psum [out rows, out cols] += ...    -> Z (row major)

Adjacent band slices overlap by ~2 output columns; PSUM's per-element
has_written accumulate handles the overlap inside one accumulation group
(start=True on the first matmul clears the bank).

Z stays in SBUF as the next level's input and is DMA'd out per level.
"""

import numpy as np

P = 128
BW = 66  # band slab width (128 input rows reach <= 66 output rows)
NPLANES = 6  # 2 images x 3 channels per core
H0 = 1024
NCORES = 8

_CACHE = {}


def _band_tables(np_dtype):
    """Banded blur+downsample matrices, sliced per 128-row tile.

    Returns (band [128, 14*66] array, offs: H -> (slab base index, [col off per k])).
    band slab s=(base+k) column block [:, s*66:(s+1)*66] holds
    W_full[128k:128(k+1), ck:ck+66] where W_full[r, i] = sum_j w[j] *
    [clamp(2i+j-2, 0, H-1) == r].
    """
    w = np.array([1.0, 4.0, 6.0, 4.0, 1.0], np.float64) / 16.0
    slabs = []
    offs = {}
    for H in (1024, 512, 256):
        Hh = H // 2
        K = H // P
        Wf = np.zeros((H, Hh), np.float64)
        for i in range(Hh):
            for j in range(5):
                r = min(max(2 * i + j - 2, 0), H - 1)
                Wf[r, i] += w[j]
        base = len(slabs)
        cks = []
        for k in range(K):
            ck = min(max(64 * k - 1, 0), Hh - BW)
            cks.append(ck)
            slabs.append(Wf[P * k : P * (k + 1), ck : ck + BW])
        offs[H] = (base, cks)
    band = np.stack(slabs)  # [14, 128, 66]
    band = band.transpose(1, 0, 2).reshape(P, len(slabs) * BW)
    return np.ascontiguousarray(band).astype(np_dtype), offs


def _split_multi_waits(nc):
    """This walrus build rejects >1 sem wait per instruction. Hoist extra
    waits onto preceding same-engine NoOps (sequencers are in-order)."""
    import bass_rust
    import concourse.mybir as mybir

    n = 0
    for func in nc.m.functions:
        for block in func.blocks:
            out = []
            changed = False
            for inst in block.instructions:
                si = inst.sync_info
                if si is not None and si.on_wait and len(si.on_wait) > 1:
                    waits = list(si.on_wait)
                    for wt in waits[:-1]:
                        n += 1
                        nop = mybir.InstNoOp(name=f"waitsplit_{n}", ins=[], outs=[])
                        nop.engine = inst.engine
                        nop.sync_info = bass_rust.SyncInfo(on_wait=[wt], on_update=[])
                        out.append(nop)
                    si.on_wait = [waits[-1]]
                    changed = True
                out.append(inst)
            if changed:
                block.instructions = out
    return n


def _build(use_bf16, loop_reps=1, variant='full'):
    import concourse.bass as bass
    import concourse.mybir as mybir
    from concourse.tile import TileContext

    if use_bf16 == "bf16" or use_bf16 is True:
        import ml_dtypes

        np_in = ml_dtypes.bfloat16
        dt_in = mybir.dt.bfloat16
    elif use_bf16 == "f32r":
        np_in = np.float32
        dt_in = mybir.dt.float32r
    else:
        np_in = np.float32
        dt_in = mybir.dt.float32
    f32 = mybir.dt.float32

    band_np, offs = _band_tables(np_in)
    nbands = band_np.shape[1] // BW

    nc = bass.Bass(enable_partition_id=False)
    x = nc.dram_tensor("x", [NPLANES, H0, H0], dt_in, kind="ExternalInput")
    wb = nc.dram_tensor("wb", [P, nbands * BW], dt_in, kind="ExternalInput")
    # merged partition-major output: per plane one [128, 2688] store
    # cols [0:2048] = L1 as [q=4, 512], [2048:2560] = L2 as [q=2, 256],
    # [2560:2688] = L3 as [q=1, 128]
    ZOFF = {1: 0, 2: 2048, 3: 2560}
    lout = nc.dram_tensor("lout", [NPLANES, P, 2688], f32, kind="ExternalOutput")

    copy_ctr = [0]

    with TileContext(nc) as tc:
        with (
            tc.tile_pool(name="consts", bufs=1) as cpool,
            tc.tile_pool(name="sbuf", bufs=2) as pool,
            tc.tile_pool(name="psum", bufs=3, space="PSUM") as ppool,
        ):
            wbt = cpool.tile([P, nbands * BW], dt_in, name="wbt")
            nc.sync.dma_start(out=wbt, in_=wb[:, :])

            def band_slice(H, k):
                base, _ = offs[H]
                s = base + k
                return wbt[:, s * BW : (s + 1) * BW]

            def copy(dst, src):
                # alternate PSUM->SBUF evacuation between ScalarE and VectorE
                copy_ctr[0] += 1
                if copy_ctr[0] % 2 == 0:
                    nc.scalar.copy(out=dst, in_=src)
                else:
                    nc.vector.tensor_copy(out=dst, in_=src)

            pe_xts = []
            if variant == "pe":
                for i in range(2):
                    t = cpool.tile([P, H0 // P, H0], dt_in, name=f"pext{i}")
                    nc.vector.memset(t, 0.0)
                    pe_xts.append(t)

            def emit_plane(p):
                if variant == "pe":
                    xt = pe_xts[p % 2]
                else:
                    xt = pool.tile([P, H0 // P, H0], dt_in, name="xt", tag="xt", bufs=2)
                    nc.sync.dma_start(
                        out=xt, in_=x[p].rearrange("(k q) w -> q k w", q=P)
                    )

                zall = pool.tile([P, 2688], f32, name="zall", tag="zall", bufs=2)

                # cur_slice(k, m) -> lhsT [128 rows of tile k, col chunk m]
                def xt_slice(k, m):
                    return xt[:, k, m * P : (m + 1) * P]

                cur_slice = xt_slice
                H = H0
                for lvl in (1, 2, 3):
                    Hh = H // 2
                    K = H // P  # input row tiles == input col chunks
                    Q = Hh // P  # output row chunks
                    _, cks = offs[H]

                    # V pass: Y^T[c, i] per 128-col chunk m
                    yt = pool.tile(
                        [P, K, Hh], dt_in, name=f"yt{lvl}", tag=f"yt{lvl}", bufs=2
                    )
                    for m in range(K):
                        ps = ppool.tile(
                            [P, Hh], f32, name=f"vps{lvl}_{m}", tag="vps", bufs=3
                        )
                        for k in range(K):
                            ck = cks[k]
                            nc.tensor.matmul(
                                ps[:, ck : ck + BW],
                                lhsT=cur_slice(k, m),
                                rhs=band_slice(H, k),
                                start=(k == 0),
                                stop=(k == K - 1),
                                skip_group_check=True,
                            )
                        copy(yt[:, m, :], ps)

                    # H pass: Z[i, j] per 128-out-row chunk q
                    zoff = ZOFF[lvl]
                    if use_bf16 and lvl < 3:
                        ztb = pool.tile(
                            [P, Q, Hh], dt_in, name=f"ztb{lvl}", tag=f"ztb{lvl}", bufs=2
                        )
                    else:
                        ztb = None
                    for q in range(Q):
                        ps = ppool.tile(
                            [P, Hh], f32, name=f"hps{lvl}_{q}", tag="hps", bufs=3
                        )
                        for m in range(K):
                            cm = cks[m]
                            nc.tensor.matmul(
                                ps[:, cm : cm + BW],
                                lhsT=yt[:, m, q * P : (q + 1) * P],
                                rhs=band_slice(H, m),
                                start=(m == 0),
                                stop=(m == K - 1),
                                skip_group_check=True,
                            )
                        copy(zall[:, zoff + q * Hh : zoff + (q + 1) * Hh], ps)
                        if ztb is not None:
                            copy(ztb[:, q, :], ps)

                    if use_bf16:
                        nxt = ztb

                        def cur_slice(k, m, _t=nxt):
                            return _t[:, k, m * P : (m + 1) * P]
                    else:

                        def cur_slice(k, m, _o=zoff, _h=Hh):
                            return zall[:, _o + k * _h + m * P : _o + k * _h + (m + 1) * P]

                    H = Hh

                if variant != "pe":
                    nc.sync.dma_start(out=lout[p], in_=zall)

            if loop_reps > 1:
                with tc.For_i(0, loop_reps, 1):
                    for p in range(NPLANES):
                        emit_plane(p)
            else:
                for p in range(NPLANES):
                    emit_plane(p)

    _split_multi_waits(nc)
    return nc


def _get_nc(use_bf16):
    key = ("nc", use_bf16)
    if key not in _CACHE:
        _CACHE[key] = _build(use_bf16)
    return _CACHE[key]


def kernel(im, levels, use_bf16=False, _want_results_obj=False, **run_kwargs):
    import ml_dtypes

    levels = int(levels)
    im = np.asarray(im)
    assert im.shape == (16, 3, H0, H0), im.shape
    assert 1 <= levels <= 4, levels

    im_f32 = np.ascontiguousarray(im, dtype=np.float32)
    outs = [im_f32]
    if levels == 1:
        return tuple(outs)

    from concourse.bass_utils import run_bass_kernel_spmd

    nc = _get_nc(use_bf16)
    np_in = ml_dtypes.bfloat16 if use_bf16 else np.float32
    band_np, _ = _band_tables(np_in)

    in_maps = []
    for c in range(NCORES):
        xc = np.ascontiguousarray(
            im_f32[2 * c : 2 * c + 2].reshape(NPLANES, H0, H0)
        ).astype(np_in)
        in_maps.append({"x": xc, "wb": band_np})

    res = run_bass_kernel_spmd(nc, in_maps, core_ids=list(range(NCORES)), **run_kwargs)

    ZOFF = {1: (0, 512, 4), 2: (2048, 256, 2), 3: (2560, 128, 1)}
    for lvl in (1, 2, 3):
        if levels <= lvl:
            break
        off, hh, q = ZOFF[lvl]
        full = np.empty((16, 3, hh, hh), np.float32)
        for c in range(NCORES):
            arr = res.results[c]["lout"][:, :, off : off + q * hh]
            # [6, 128, q, hh] -> [6, q, 128, hh] -> [6, hh, hh]
            planes = (
                arr.reshape(NPLANES, P, q, hh)
                .transpose(0, 2, 1, 3)
                .reshape(NPLANES, q * P, hh)
            )
            full[2 * c : 2 * c + 2] = planes.reshape(2, 3, hh, hh)
        outs.append(full)

    if _want_results_obj:
        return tuple(outs[:levels]), res
    return tuple(outs[:levels])
